# revision 7
# baseline (speedup 1.0000x reference)
"""Trainium2 Bass kernel for nn_DCM_22058952032783 (dynamic-conv CNN).

Strategy: data-parallel over batch (B=8 -> 8 NeuronCores) + fp8 DoubleRow
matmuls (2x PE throughput) for every GEMM whose quantization error fits the
budget (measured offline: tr convs, G convs, depthwise taps OK at ~3e-3
scale-relative; the fused C1/fo_x pass must stay fp16).

Per-sample math (reference):
    feats = [x]
    for k in (1, 3, 5):
        pooled = adaptive_avg_pool(y, k)               # [inC, k, k]
        kern   = gk_w @ pooled + gk_b                  # [mid, k*k] dyn kernels
        x_in   = tr_w @ x + tr_b                       # [mid, HW]
        dwout  = depthwise(x_in, kern)                 # [mid, HW]
        feats.append(fi_w @ dwout + fi_b)
    out = fo_w @ concat(feats) + fo_b

Host-side exact algebraic folds (as v1):
  - G_i = fo_blk_i @ fi_w_i folds fi into fo; b' = fo_b + sum fo_blk_i@fi_b_i
  - k=1 branch folded into C1 = fo_x + tr1^T @ (G1 * kern1), built on device.

Device pipeline per core:
  - pooling via fp8 DR indicator matmul, kern-gen fp16 (tiny)
  - tr convs: fp8 DR GEMMs -> PSUM -> dual eviction (xpad fp16 for DVE taps,
    xpad fp8 for PE taps), zero-padded images
  - depthwise: N_PE taps as fp8 DR tap-PAIR diag matmuls (2 taps/instr via a
    custom-stride 4-D rhs AP), the rest as DVE tensor_scalar+tensor_tensor
  - acc16 (scale 64) -> acc8, G5/G3 fp8 DR GEMMs folded into the fp16
    partial (C1x pass output); G5 interleaves with the k3 units
  - output fp16, widened to fp32 on host

Scales (all powers of 2, folded into weights/biases/eviction scales):
  x8 = x*16, y8 = y*16, trT8 = tr^T*64, G8 = G^T*256, xpad = x_in*32,
  kern tile = kern*2, diag8 = kern*64 (eye32 trick), acc16/acc8 = acc*64.
"""

import os

import numpy as np

# ---- hardcoded problem shapes (nn_DCM_22058952032783) ----
B, inC, midC, outC, H, W, Hy = 8, 512, 512, 512, 64, 64, 60
HW = H * W            # 4096
P = 128
KC = inC // P         # 4 partition chunks of the channel dims
HALF = 2048           # psum half (4 banks)
PAD = 2
PR = H + 2 * PAD      # 68 padded rows
PC = W + 2 * PAD      # 68 padded cols
N_CORES = 8

# column layout of the fused pooled/kern tiles: [k5 (25) | k3 (9) | k1 (1)]
OFF5, OFF3, OFF1 = 0, 25, 34
CNT5, CNT3, CNT1 = (Hy // 5) ** 2, (Hy // 3) ** 2, Hy * Hy  # 144, 400, 3600
YCH = 30              # zero-padded pixel chunks of transposed y (3840/128)

# taps 0..N_PE-1 run on the PE as fp8 DoubleRow pairs; the rest on DVE.
N_PE5 = 14            # of 25 (even)
N_PE3 = 6             # of 9  (even)

# fp32 per-partition bias/scale tile [128, 29] column layout
TRB5, TRB3, GKB1, GKB3, GKB5, BFO, OB, PS = 0, 4, 8, 12, 16, 20, 24, 28

_CACHED = {}


def _build_program():
    import concourse.bacc as bacc
    import concourse.mybir as mybir
    import concourse.tile as tile

    fp8 = mybir.dt.float8e4
    fp16 = mybir.dt.float16
    fp32 = mybir.dt.float32

    nc = bacc.Bacc("TRN2", debug=False)

    din = {}

    def ext_in(name, shape, dt):
        din[name] = nc.dram_tensor(name, shape, dt, kind="ExternalInput").ap()
        return din[name]

    ext_in("x16", [P, KC, HW], fp16)
    ext_in("x8", [P, KC, HW], fp8)
    ext_in("yT8", [P, YCH, inC], fp8)
    ext_in("pm8", [P, YCH, 64], fp8)
    ext_in("trT8_5", [P, 2, 2, midC], fp8)
    ext_in("trT8_3", [P, 2, 2, midC], fp8)
    ext_in("GT8_5", [P, 2, 2, outC], fp8)
    ext_in("GT8_3", [P, 2, 2, outC], fp8)
    ext_in("gkT1", [P, KC, midC], fp16)
    ext_in("gkT3", [P, KC, midC], fp16)
    ext_in("gkT5", [P, KC, midC], fp16)
    ext_in("tr1", [P, KC, inC], fp16)
    ext_in("G1T", [P, KC, outC], fp16)
    ext_in("foxT", [P, KC, outC], fp16)
    ext_in("b1", [P, KC, 1], fp16)
    ext_in("biases", [P, 29], fp32)
    ext_in("eye16", [P, P], fp16)
    ext_in("eye32", [P, P], fp16)

    outd = nc.dram_tensor("out", [P, KC, HW], fp16, kind="ExternalOutput").ap()

    with tile.TileContext(nc) as tc:
        _emit(nc, tc, mybir, din, outd, fp8, fp16, fp32)
    nc.compile()
    return nc


def _emit(nc, tc, mybir, din, outd, fp8, fp16, fp32):
    from contextlib import ExitStack

    Alu = mybir.AluOpType
    AF = mybir.ActivationFunctionType
    DR = mybir.MatmulPerfMode.DoubleRow

    def pair_rhs(pv, dy0, dx0, delta, r0, nrow):
        v = pv[:, PAD + dy0 + r0:PAD + dy0 + r0 + nrow,
               PAD + dx0:PAD + dx0 + W]
        v = v.unsqueeze(1).to_broadcast([P, 2, nrow, W])
        v.ap[1] = (delta, 2)
        return v

    ctx = ExitStack()
    with ctx:
        wmain = ctx.enter_context(tc.tile_pool(name="wmain", bufs=1))
        psum = ctx.enter_context(tc.tile_pool(name="psum", bufs=2,
                                              space="PSUM"))

        # ---------- persistent tiles ----------
        xs16 = wmain.tile([P, KC, HW], fp16, tag="xs16")
        xs8 = wmain.tile([P, KC, HW], fp8, tag="xs8")
        kern = wmain.tile([P, KC, 36], fp32, tag="kern")
        biases = wmain.tile([P, 29], fp32, tag="biases")
        w_tr8 = {5: wmain.tile([P, 2, 2, midC], fp8, tag="trT8_5",
                               name="trT8_5"),
                 3: wmain.tile([P, 2, 2, midC], fp8, tag="trT8_3",
                               name="trT8_3")}
        w_G8 = {5: wmain.tile([P, 2, 2, outC], fp8, tag="GT8_5",
                              name="GT8_5"),
                3: wmain.tile([P, 2, 2, outC], fp8, tag="GT8_3",
                              name="GT8_3")}
        w_C1T = wmain.tile([P, KC, outC], fp16, tag="C1T")
        acc8 = {5: wmain.tile([P, KC, HW], fp8, tag="acc8_5", name="acc8_5"),
                3: wmain.tile([P, KC, HW], fp8, tag="acc8_3", name="acc8_3")}
        partial = wmain.tile([P, KC, HW], fp16, tag="partial")
        eye16 = wmain.tile([P, P], fp16, tag="eye16")
        eye32 = wmain.tile([P, P], fp16, tag="eye32")

        # two alternating padded-image sets (borders memset once)
        xp16 = [wmain.tile([P, PR, PC], fp16, tag=f"xp16_{i}",
                           name=f"xp16_{i}") for i in range(2)]
        xp8 = [wmain.tile([P, PR, PC], fp8, tag=f"xp8_{i}",
                          name=f"xp8_{i}") for i in range(2)]
        for t in xp16 + xp8:
            nc.gpsimd.memset(t[:, 0:PAD, :], 0.0)
            nc.gpsimd.memset(t[:, PAD + H:PR, :], 0.0)
            nc.gpsimd.memset(t[:, PAD:PAD + H, 0:PAD], 0.0)
            nc.gpsimd.memset(t[:, PAD:PAD + H, PC - PAD:PC], 0.0)

        def emit_branch_tr(kk, m, trb, u):
            """fp8 DR tr conv for one mid-chunk -> padded x_in images
            (fp16 for DVE taps + fp8 for PE taps)."""
            w = w_tr8[kk]
            p16, p8 = xp16[u % 2], xp8[u % 2]
            for half in range(2):
                ps = psum.tile([P, HALF], fp32, tag="ps",
                               name=f"pstr_{kk}_{m}_{half}")
                for kcp in range(2):
                    for nb in range(8):
                        c0 = half * HALF + nb * 256
                        nc.tensor.matmul(
                            ps[:, nb * 256:(nb + 1) * 256],
                            w[:, kcp, :, m * P:(m + 1) * P],
                            xs8[:, 2 * kcp:2 * kcp + 2, c0:c0 + 256],
                            start=(kcp == 0 and nb % 2 == 0),
                            stop=(kcp == 1 and nb % 2 == 1),
                            perf_mode=DR)
                psv = ps[:].rearrange("p (r c) -> p r c", c=W)
                r0 = PAD + half * (H // 2)
                nc.scalar.activation(
                    p16[:, r0:r0 + H // 2, PAD:PAD + W], psv,
                    AF.Identity, bias=biases[:, trb + m:trb + m + 1],
                    scale=2.0 ** -5)
                nc.scalar.copy(p8[:, r0:r0 + H // 2, PAD:PAD + W],
                               p16[:, r0:r0 + H // 2, PAD:PAD + W])
            return p16, p8

        # ---------- stage 1: pooling + kernel generation + C1 ----------
        premade = {}
        with tc.tile_pool(name="wtmp", bufs=1) as wtmp:
            yT8 = wtmp.tile([P, YCH, inC], fp8, tag="yT8")
            pm8 = wtmp.tile([P, YCH, 64], fp8, tag="pm8")
            pooled = wtmp.tile([P, KC, 35], fp16, tag="pooled")
            pooledT = wtmp.tile([35, inC], fp16, tag="pooledT")
            w_gk = [wtmp.tile([P, KC, midC], fp16, tag=f"gkT{k}",
                              name=f"gkT{k}") for k in (1, 3, 5)]
            w_tr1 = wtmp.tile([P, KC, inC], fp16, tag="tr1")
            w_G1T = wtmp.tile([P, KC, outC], fp16, tag="G1T")
            w_foxT = wtmp.tile([P, KC, outC], fp16, tag="foxT")
            w_Gs1T = wtmp.tile([P, KC, outC], fp16, tag="Gs1T")
            b1 = wtmp.tile([P, KC, 1], fp16, tag="b1")

            # DMA queues: scalar feeds the first tr GEMMs, sync the pooling
            # chain, gpsimd the rest.
            nc.scalar.dma_start(w_tr8[5][:], din["trT8_5"][:])
            for kc in range(KC):
                nc.scalar.dma_start(xs8[:, kc], din["x8"][:, kc])
            nc.scalar.dma_start(w_tr8[3][:], din["trT8_3"][:])
            for kc in range(KC):
                nc.gpsimd.dma_start(xs16[:, kc], din["x16"][:, kc])
            nc.sync.dma_start(pm8[:], din["pm8"][:])
            nc.sync.dma_start(yT8[:], din["yT8"][:])
            for t, n in ((w_gk[2], "gkT5"), (w_gk[1], "gkT3"),
                         (w_gk[0], "gkT1"), (w_tr1, "tr1"), (w_G1T, "G1T"),
                         (w_foxT, "foxT"), (b1, "b1")):
                nc.sync.dma_start(t[:], din[n][:])
            for t, n in ((biases, "biases"), (eye16, "eye16"),
                         (eye32, "eye32"), (w_G8[5], "GT8_5"),
                         (w_G8[3], "GT8_3")):
                nc.gpsimd.dma_start(t[:], din[n][:])

            # first k5 units' tr convs ahead of the pooling chain
            premade[(5, 0)] = emit_branch_tr(5, 0, TRB5, 0)
            premade[(5, 1)] = emit_branch_tr(5, 1, TRB5, 1)

            # pooling on PE via fp8 DR: pooledT[j, c] = sum_px pm[px,j] y[px,c]
            psp = psum.tile([P, HALF], fp32, tag="ps", name="psp")
            for chp in range(YCH // 2):
                for blk in range(2):
                    nc.tensor.matmul(
                        psp[:64, blk * 256:(blk + 1) * 256],
                        pm8[:, 2 * chp:2 * chp + 2, :],
                        yT8[:, 2 * chp:2 * chp + 2,
                            blk * 256:(blk + 1) * 256],
                        start=(chp == 0 and blk == 0),
                        stop=(chp == YCH // 2 - 1 and blk == 1),
                        perf_mode=DR)
            # pooled means: per-partition (=block index) 1/(Sy*cnt) scale
            nc.scalar.activation(pooledT[:], psp[:35, :inC], AF.Identity,
                                 scale=biases[:35, PS:PS + 1])
            for m in range(KC):
                pst = psum.tile([P, 64], fp16, tag="ps", name=f"pst{m}")
                nc.tensor.transpose(pst[:, :35],
                                    pooledT[:, m * P:(m + 1) * P],
                                    eye16[:35, :35])
                nc.vector.tensor_copy(pooled[:, m, :], pst[:, :35])

            # kern = gk_w @ pooled + gk_b (gkT3/5 and their biases pre-scaled
            # by 2 host-side, so kern cols hold 2*kern for k3/k5, exact k1)
            for w_g, off, kk2, gb in ((w_gk[2], OFF5, 25, GKB5),
                                      (w_gk[1], OFF3, 9, GKB3),
                                      (w_gk[0], OFF1, 1, GKB1)):
                for m in range(KC):
                    ps = psum.tile([P, HALF], fp32, tag="ps")
                    for kc in range(KC):
                        nc.tensor.matmul(
                            ps[:, :kk2],
                            w_g[:, kc, m * P:(m + 1) * P],
                            pooled[:, kc, off:off + kk2],
                            start=(kc == 0), stop=(kc == KC - 1))
                    nc.vector.tensor_scalar(
                        kern[:, m, off:off + kk2], ps[:, :kk2],
                        biases[:, gb + m:gb + m + 1], None, op0=Alu.add)

            # k1 branch folded matrix: C1T = foxT + (tr1^T @ (G1T*kern1))
            for kc in range(KC):
                nc.vector.tensor_scalar_mul(
                    w_Gs1T[:, kc, :], w_G1T[:, kc, :],
                    kern[:, kc, OFF1:OFF1 + 1])
            for mi in range(KC):
                ps = psum.tile([P, HALF], fp32, tag="ps")
                for kc in range(KC):
                    nc.tensor.matmul(
                        ps[:, :outC], w_tr1[:, kc, mi * P:(mi + 1) * P],
                        w_Gs1T[:, kc, :],
                        start=(kc == 0), stop=(kc == KC - 1))
                nc.vector.tensor_tensor(
                    w_C1T[:, mi, :], ps[:, :outC], w_foxT[:, mi, :],
                    op=Alu.add)
            # out bias = b' + G1 @ (kern1 * b1)
            for m in range(KC):
                ps = psum.tile([P, HALF], fp32, tag="ps")
                for kc in range(KC):
                    nc.tensor.matmul(
                        ps[:, :1], w_Gs1T[:, kc, m * P:(m + 1) * P],
                        b1[:, kc, :],
                        start=(kc == 0), stop=(kc == KC - 1))
                nc.vector.tensor_tensor(
                    biases[:, OB + m:OB + m + 1], ps[:, :1],
                    biases[:, BFO + m:BFO + m + 1], op=Alu.add)

        # ---------- stage 2: units + fused fo ----------
        dtmp = ctx.enter_context(tc.tile_pool(name="dtmp", bufs=2))
        diagp = ctx.enter_context(tc.tile_pool(name="diagp", bufs=2))
        accp = ctx.enter_context(tc.tile_pool(name="accp", bufs=2))
        evp = ctx.enter_context(tc.tile_pool(name="evp", bufs=2))
        outp = ctx.enter_context(tc.tile_pool(name="outp", bufs=2))

        def emit_c1x_group(g):
            m, half = g // 2, g % 2
            ps = psum.tile([P, HALF], fp32, tag="ps", name=f"psc1_{g}")
            for kc in range(KC):
                for nb in range(4):
                    nc.tensor.matmul(
                        ps[:, nb * 512:(nb + 1) * 512],
                        w_C1T[:, kc, m * P:(m + 1) * P],
                        xs16[:, kc, half * HALF + nb * 512:
                             half * HALF + (nb + 1) * 512],
                        start=(kc == 0), stop=(kc == KC - 1))
            nc.scalar.activation(
                partial[:, m, half * HALF:(half + 1) * HALF], ps[:],
                AF.Identity, bias=biases[:, OB + m:OB + m + 1])

        def emit_G_group(kk, mo, half, last=False):
            """G @ acc8 for one (mo, half): DR GEMM -> evict -> add."""
            w = w_G8[kk]
            rhs = acc8[kk]
            ps = psum.tile([P, HALF], fp32, tag="ps", name=f"psG{kk}_{mo}_{half}")
            for kcp in range(2):
                for nb in range(8):
                    c0 = half * HALF + nb * 256
                    nc.tensor.matmul(
                        ps[:, nb * 256:(nb + 1) * 256],
                        w[:, kcp, :, mo * P:(mo + 1) * P],
                        rhs[:, 2 * kcp:2 * kcp + 2, c0:c0 + 256],
                        start=(kcp == 0 and nb % 2 == 0),
                        stop=(kcp == 1 and nb % 2 == 1),
                        perf_mode=DR)
            tmp = evp.tile([P, HALF], fp16, tag="gtmp",
                           name=f"gtmp{kk}_{mo}_{half}")
            nc.scalar.activation(tmp[:], ps[:], AF.Identity, scale=2.0 ** -14)
            pslice = partial[:, mo, half * HALF:(half + 1) * HALF]
            if last:
                outs = outp.tile([P, HALF], fp16, tag="outs",
                                 name=f"outs{mo}_{half}")
                nc.gpsimd.tensor_tensor(outs[:], tmp[:], pslice, op=Alu.add)
                nc.sync.dma_start(
                    outd[:, mo, half * HALF:(half + 1) * HALF], outs[:])
            else:
                nc.vector.tensor_tensor(pslice, tmp[:], pslice, op=Alu.add)

        units = [(5, m) for m in range(KC)] + [(3, m) for m in range(KC)]
        g5_queue = [(mo, half) for mo in range(KC) for half in range(2)]

        for u, (kk, m) in enumerate(units):
            koff = OFF5 if kk == 5 else OFF3
            trb = TRB5 if kk == 5 else TRB3
            n_pe = N_PE5 if kk == 5 else N_PE3
            p = kk // 2

            def off(t):
                return (t // kk - p) * PC + (t % kk - p)

            if (kk, m) in premade:
                p16, p8 = premade.pop((kk, m))
            else:
                p16, p8 = emit_branch_tr(kk, m, trb, u)

            # diag pair matrices for PE taps: diag8 = eye32 * (2*kern)
            diags = diagp.tile([P, N_PE5, P], fp8, tag="diags")
            nc.vector.tensor_tensor(
                diags[:, 0:n_pe, :],
                eye32[:].unsqueeze(1).to_broadcast([P, n_pe, P]),
                kern[:, m, koff:koff + n_pe].unsqueeze(2).to_broadcast(
                    [P, n_pe, P]),
                op=Alu.mult)

            acc16 = accp.tile([P, H, W], fp16, tag="acc16", name=f"acc16_{u}")
            av8 = acc8[kk][:, m].rearrange("p (h w) -> p h w", w=W)
            for half in range(2):
                # PE taps: tap-pair DR diag matmuls accumulate in PSUM
                psd = psum.tile([P, HALF], fp32, tag="ps", name=f"psd{u}{half}")
                for a in range(n_pe // 2):
                    t0 = 2 * a
                    dy0, dx0 = t0 // kk - p, t0 % kk - p
                    delta = off(t0 + 1) - off(t0)
                    for nb in range(8):
                        r0 = half * 32 + nb * 4
                        nc.tensor.matmul(
                            psd[:, nb * 256:(nb + 1) * 256],
                            diags[:, t0:t0 + 2, :],
                            pair_rhs(p8, dy0, dx0, delta, r0, 4),
                            start=(a == 0 and nb % 2 == 0),
                            stop=(a == n_pe // 2 - 1 and nb % 2 == 1),
                            perf_mode=DR)
                # evict = init of acc16 (psum holds 64*dw_part; scale 2^-5)
                nc.scalar.activation(
                    acc16[:, half * 32:half * 32 + 32, :],
                    psd[:].rearrange("p (r c) -> p r c", c=W),
                    AF.Identity, scale=2.0 ** -5)
                # DVE taps for this half
                for t in range(n_pe, kk * kk):
                    dy, dx = t // kk - p, t % kk - p
                    r0 = half * 32
                    tmp = dtmp.tile([P, 32, W], fp16, tag="dvetmp")
                    nc.vector.tensor_scalar_mul(
                        tmp[:],
                        p16[:, PAD + dy + r0:PAD + dy + r0 + 32,
                            PAD + dx:PAD + dx + W],
                        kern[:, m, koff + t:koff + t + 1])
                    avh = acc16[:, r0:r0 + 32, :]
                    nc.vector.tensor_tensor(avh, tmp[:], avh, op=Alu.add)
                # acc8 conversion for the G GEMM rhs
                nc.scalar.activation(
                    av8[:, half * 32:half * 32 + 32, :],
                    acc16[:, half * 32:half * 32 + 32, :], AF.Identity)

            emit_c1x_group(u)
            if kk == 3:
                # interleave two G5 groups per k3 unit
                emit_G_group(5, *g5_queue[2 * (u - KC)])
                emit_G_group(5, *g5_queue[2 * (u - KC) + 1])

        # ---------- tail: G3 + final out ----------
        for mo in range(KC):
            for half in range(2):
                emit_G_group(3, mo, half, last=True)


def _chunk_pm(a):
    """[512, F...] -> [128, 4, F...] partition-major chunks."""
    return np.ascontiguousarray(
        a.reshape(KC, P, *a.shape[1:]).transpose(1, 0, *range(2, a.ndim + 1)))


def _pair_chunk(a):
    """[512, F] -> [128, 2, 2, F]: (kc pair, within-pair) for DR lhsT."""
    return np.ascontiguousarray(
        a.reshape(2, 2, P, a.shape[1]).transpose(2, 0, 1, 3))


def _prep_host(inputs):
    import ml_dtypes
    f8 = ml_dtypes.float8_e4m3
    f16 = np.float16
    f32 = np.float32
    x = np.asarray(inputs["x"], f32)
    y = np.asarray(inputs["y"], f32)
    gk_w = np.asarray(inputs["gk_w"], f32)
    gk_b = np.asarray(inputs["gk_b"], f32)
    tr_w = np.asarray(inputs["tr_w"], f32)
    tr_b = np.asarray(inputs["tr_b"], f32)
    fi_w = np.asarray(inputs["fi_w"], f32)
    fi_b = np.asarray(inputs["fi_b"], f32)
    fo_w = np.asarray(inputs["fo_w"], f32)
    fo_b = np.asarray(inputs["fo_b"], f32)

    fo_x = fo_w[:, :inC]
    fo_blk = [fo_w[:, inC + i * midC: inC + (i + 1) * midC] for i in range(3)]
    G = [fo_blk[i] @ fi_w[i] for i in range(3)]
    bfo = fo_b + sum(fo_blk[i] @ fi_b[i] for i in range(3))

    def q8(a, s):
        return np.clip(a * s, -240, 240).astype(f8)

    shared = {
        "trT8_5": _pair_chunk(q8(np.ascontiguousarray(tr_w[2].T), 64.0)),
        "trT8_3": _pair_chunk(q8(np.ascontiguousarray(tr_w[1].T), 64.0)),
        "GT8_5": _pair_chunk(q8(np.ascontiguousarray(G[2].T), 256.0)),
        "GT8_3": _pair_chunk(q8(np.ascontiguousarray(G[1].T), 256.0)),
        "gkT1": _chunk_pm(np.ascontiguousarray(gk_w[0].T).astype(f16)),
        "gkT3": _chunk_pm((2.0 * np.ascontiguousarray(gk_w[1].T)).astype(f16)),
        "gkT5": _chunk_pm((2.0 * np.ascontiguousarray(gk_w[2].T)).astype(f16)),
        "tr1": _chunk_pm(tr_w[0].astype(f16)),
        "G1T": _chunk_pm(np.ascontiguousarray(G[0].T).astype(f16)),
        "foxT": _chunk_pm(np.ascontiguousarray(fo_x.T).astype(f16)),
        "b1": _chunk_pm(tr_b[0].astype(f16)[:, None]),
        "eye16": np.eye(P, dtype=f16),
        "eye32": (np.eye(P, dtype=f32) * 32.0).astype(f16),
    }
    # pooling indicator matrix [3840, 35] (0/1, exact in fp8); the mean
    # normalization + y-scale live in the PS eviction-scale column
    pmat = np.zeros((YCH * P, 64), f32)
    hw_idx = np.arange(Hy * Hy)
    hh, ww = hw_idx // Hy, hw_idx % Hy
    for j in range(25):
        jh, jw = j // 5, j % 5
        pmat[:Hy * Hy, OFF5 + j] = ((hh // (Hy // 5) == jh) &
                                    (ww // (Hy // 5) == jw))
    for j in range(9):
        jh, jw = j // 3, j % 3
        pmat[:Hy * Hy, OFF3 + j] = ((hh // (Hy // 3) == jh) &
                                    (ww // (Hy // 3) == jw))
    pmat[:Hy * Hy, OFF1] = 1.0
    shared["pm8"] = np.ascontiguousarray(
        pmat.reshape(YCH, P, 64).transpose(1, 0, 2)).astype(f8)

    biases = np.zeros((P, 29), f32)
    for col, vec in ((TRB5, 32.0 * tr_b[2]), (TRB3, 32.0 * tr_b[1]),
                     (GKB1, gk_b[0]), (GKB3, 2.0 * gk_b[1]),
                     (GKB5, 2.0 * gk_b[2]), (BFO, bfo)):
        biases[:, col:col + KC] = vec.reshape(KC, P).T
    cnt = np.zeros(35, f32)
    cnt[OFF5:OFF5 + 25] = CNT5
    cnt[OFF3:OFF3 + 9] = CNT3
    cnt[OFF1] = CNT1
    biases[:35, PS] = 1.0 / (16.0 * cnt)
    shared["biases"] = biases

    per_core = []
    for b in range(B):
        yt = np.zeros((YCH * P, inC), f32)
        yt[:Hy * Hy] = y[b].reshape(inC, Hy * Hy).T * 16.0
        xb = x[b].reshape(inC, HW)
        per_core.append({
            "x16": _chunk_pm(xb.astype(f16)),
            "x8": _chunk_pm(q8(xb, 16.0)),
            "yT8": np.ascontiguousarray(
                yt.reshape(YCH, P, inC).transpose(1, 0, 2)).astype(f8),
        })
    return shared, per_core


LAST_RESULTS = None


def _ensure_ntff_hook():
    """Best-effort: recreate the missing antenv.axon_hooks module so
    run_bass_kernel_spmd(trace=True) can capture NTFF profiles under axon."""
    import sys
    import types
    try:
        from antenv.axon_hooks import get_axon_ntff_profile_hook  # noqa: F401
        return
    except ImportError:
        pass
    try:
        import antenv
        from trn_agent_boot.trn_boot import _ntff_profile_via_ctypes
        mod = types.ModuleType("antenv.axon_hooks")
        mod._hook = None

        def set_axon_ntff_profile_hook(h):
            mod._hook = h

        def get_axon_ntff_profile_hook():
            return mod._hook

        mod.set_axon_ntff_profile_hook = set_axon_ntff_profile_hook
        mod.get_axon_ntff_profile_hook = get_axon_ntff_profile_hook
        sys.modules["antenv.axon_hooks"] = mod
        antenv.axon_hooks = mod
        mod.set_axon_ntff_profile_hook(
            _ntff_profile_via_ctypes("/opt/axon/libaxon_pjrt.so"))
    except Exception as e:  # profiling is optional — never break the run
        print(f"ntff hook unavailable: {e}")


def kernel(**inputs) -> np.ndarray:
    global LAST_RESULTS
    if "nc" not in _CACHED:
        _CACHED["nc"] = _build_program()
    nc = _CACHED["nc"]

    shared, per_core = _prep_host(inputs)
    in_maps = [{**shared, **pc} for pc in per_core]

    from concourse import bass_utils
    trace = bool(os.environ.get("DCM_TRACE"))
    if trace:
        _ensure_ntff_hook()
    res = bass_utils.run_bass_kernel_spmd(
        nc, in_maps, core_ids=list(range(N_CORES)), trace=trace)
    LAST_RESULTS = res

    out = np.empty((B, outC, H, W), np.float32)
    for b in range(B):
        o = np.asarray(res.results[b]["out"], np.float32)  # [128, KC, HW]
        out[b] = o.transpose(1, 0, 2).reshape(outC, H, W)
    return out
